# revision 4
# baseline (speedup 1.0000x reference)
"""MoE (8 experts, top-5 Boltzmann gate) Trainium2 kernel.

Strategy: data-parallel over tokens. Each of the 8 NeuronCores processes
B/8 = 512 tokens and runs all 8 experts fused (hT stays in SBUF between
the two matmuls); gate weights are applied per-partition at PSUM evict.
No collectives — the host slices tokens and concatenates the outputs.

Host-side prep (sharding/layout): weights are pre-transposed and tiled so
every DMA is contiguous per partition.
"""

import numpy as np

# Problem dims (hardcoded per contract)
D_FULL, H_FULL, O_FULL, NEXP = 1024, 4096, 1024, 8
B_FULL = 4096
NCORES = 8
TEMP = float(np.e)
BIG = 1.0e30
KH_CHUNK = 8  # mm2 contraction tiles per PSUM accumulation group


def build_moe_bass(Bc, D, H, O, N, temp, num_devices=NCORES):
    """Build the per-core Bass/Tile program. Bc = tokens per core (<=512)."""
    from contextlib import ExitStack

    import concourse.bass as bass
    import concourse.tile as tile
    from concourse import bacc, mybir

    f32 = mybir.dt.float32
    P = 128
    assert Bc % P == 0 and Bc <= 512
    assert D % P == 0 and H % (2 * P) == 0 and O % 512 == 0
    KD, KH, MB, NO = D // P, H // P, Bc // P, O // 512
    MH = H // P
    KH2 = KH // 2  # kh pairs (w2 slab granularity)

    nc = bacc.Bacc(
        "TRN2", target_bir_lowering=False, debug=False, num_devices=num_devices
    )

    # DRAM I/O (host-packed layouts; all per-partition contiguous)
    xt_d = nc.dram_tensor("xt", [P, KD, Bc], f32, kind="ExternalInput").ap()
    w1_d = nc.dram_tensor("w1t", [N, MH, P, KD, P], f32, kind="ExternalInput").ap()
    w2_d = nc.dram_tensor("w2t", [N, KH2, P, 2, O], f32, kind="ExternalInput").ap()
    b1_d = nc.dram_tensor("b1p", [P, N, MH], f32, kind="ExternalInput").ap()
    b2_d = nc.dram_tensor("b2s", [N, O], f32, kind="ExternalInput").ap()
    wg_d = nc.dram_tensor("wgt", [P, KD, N], f32, kind="ExternalInput").ap()
    bg_d = nc.dram_tensor("bgr", [P, N], f32, kind="ExternalInput").ap()
    out_d = nc.dram_tensor("out", [Bc, O], f32, kind="ExternalOutput").ap()

    Exp = mybir.ActivationFunctionType.Exp
    Relu = mybir.ActivationFunctionType.Relu
    Alu = mybir.AluOpType

    with tile.TileContext(nc) as tc, ExitStack() as ctx:
        const = ctx.enter_context(tc.tile_pool(name="const", bufs=1))
        gatep = ctx.enter_context(tc.tile_pool(name="gate", bufs=2))
        xtp = ctx.enter_context(tc.tile_pool(name="xt", bufs=1))
        w1p = ctx.enter_context(tc.tile_pool(name="w1", bufs=3))
        w2p = ctx.enter_context(tc.tile_pool(name="w2", bufs=6))
        htp = ctx.enter_context(tc.tile_pool(name="ht", bufs=MH + 1))
        accp = ctx.enter_context(tc.tile_pool(name="acc", bufs=MB))
        ps_s = ctx.enter_context(tc.tile_pool(name="ps_s", bufs=2, space="PSUM"))
        ps_1 = ctx.enter_context(tc.tile_pool(name="ps_1", bufs=3, space="PSUM"))
        ps_2 = ctx.enter_context(tc.tile_pool(name="ps_2", bufs=3, space="PSUM"))

        # ---- constant / input loads ----
        xt = xtp.tile([P, KD, Bc], f32)
        nc.sync.dma_start(xt[:], xt_d[:])
        wg_sb = const.tile([P, KD, N], f32)
        nc.sync.dma_start(wg_sb[:], wg_d[:])
        bg_sb = const.tile([P, N], f32)
        nc.sync.dma_start(bg_sb[:], bg_d[:])
        b1_sb = const.tile([P, N, MH], f32)
        nc.sync.dma_start(b1_sb[:], b1_d[:])
        b2_sb = const.tile([N, O], f32)
        nc.sync.dma_start(b2_sb[:], b2_d[:])

        w_sb = const.tile([P, MB, N], f32)  # gate weights, [token_p, mtile, expert]
        wt_sb = const.tile([32, Bc], f32)  # transposed gate weights (rows 0..N-1)

        # ---- gate: logits -> softmax(l/T) -> top-5 mask -> renormalize ----
        for m in range(MB):
            pg = ps_s.tile([P, N], f32, tag="ps_small")
            for k in range(KD):
                nc.tensor.matmul(
                    pg[:],
                    xt[:, k, m * P : (m + 1) * P],
                    wg_sb[:, k, :],
                    start=(k == 0),
                    stop=(k == KD - 1),
                )
            lg = gatep.tile([P, N], f32, tag="g_l")
            nc.vector.tensor_tensor(lg[:], pg[:], bg_sb[:], Alu.add)
            rmax = gatep.tile([P, 1], f32, tag="g_max")
            nc.vector.reduce_max(rmax[:], lg[:], axis=mybir.AxisListType.X)
            nbias = gatep.tile([P, 1], f32, tag="g_nb")
            nc.scalar.mul(nbias[:], rmax[:], -1.0 / temp)
            e = gatep.tile([P, N], f32, tag="g_e")
            nc.scalar.activation(e[:], lg[:], Exp, bias=nbias[:], scale=1.0 / temp)
            z = gatep.tile([P, 1], f32, tag="g_z")
            nc.vector.reduce_sum(z[:], e[:], axis=mybir.AxisListType.X)
            zi = gatep.tile([P, 1], f32, tag="g_zi")
            nc.vector.reciprocal(zi[:], z[:])
            p = gatep.tile([P, N], f32, tag="g_p")
            nc.vector.tensor_scalar_mul(p[:], e[:], zi[:])
            # 3rd-smallest per row via iterated min-masking (drop bottom N-NA=3)
            cur = p
            mn = None
            for r in range(3):
                mn = gatep.tile([P, 1], f32, tag=f"g_mn{r}")
                nc.vector.tensor_reduce(
                    mn[:], cur[:], axis=mybir.AxisListType.X, op=Alu.min
                )
                if r < 2:
                    msk = gatep.tile([P, N], f32, tag=f"g_msk{r}")
                    nc.vector.tensor_scalar(
                        msk[:], cur[:], mn[:], BIG, op0=Alu.is_equal, op1=Alu.mult
                    )
                    nxt = gatep.tile([P, N], f32, tag=f"g_nxt{r}")
                    nc.vector.tensor_tensor(nxt[:], msk[:], cur[:], Alu.max)
                    cur = nxt
            pm = gatep.tile([P, N], f32, tag="g_pm")
            nc.vector.scalar_tensor_tensor(
                pm[:], p[:], mn[:], p[:], op0=Alu.is_gt, op1=Alu.mult
            )
            s = gatep.tile([P, 1], f32, tag="g_s")
            nc.vector.reduce_sum(s[:], pm[:], axis=mybir.AxisListType.X)
            se = gatep.tile([P, 1], f32, tag="g_se")
            nc.vector.tensor_scalar_add(se[:], s[:], 1.0e-8)
            si = gatep.tile([P, 1], f32, tag="g_si")
            nc.vector.reciprocal(si[:], se[:])
            nc.vector.tensor_scalar_mul(w_sb[:, m, :], pm[:], si[:])

            # transpose this m-tile's gate weights into wt_sb[0:N, m*P:(m+1)*P]
            wpad = gatep.tile([P, 32], f32, tag="g_wpad")
            nc.vector.memset(wpad[:], 0.0)
            nc.vector.tensor_copy(wpad[:, 0:N], w_sb[:, m, :])
            for blk in range(4):
                nc.vector.transpose(
                    wt_sb[0:32, m * P + 32 * blk : m * P + 32 * (blk + 1)],
                    wpad[32 * blk : 32 * (blk + 1), 0:32],
                )

        # ---- out_acc init: b2 contribution = w @ b2_stack (K = N experts) ----
        acc = [
            accp.tile([P, O], f32, name=f"acc{m}", tag="acc") for m in range(MB)
        ]
        for m in range(MB):
            for o2 in range(NO):
                pb = ps_s.tile([P, 512], f32, tag="ps_small")
                nc.tensor.matmul(
                    pb[:],
                    wt_sb[0:N, m * P : (m + 1) * P],
                    b2_sb[0:N, o2 * 512 : (o2 + 1) * 512],
                    start=True,
                    stop=True,
                )
                nc.vector.tensor_copy(acc[m][:, o2 * 512 : (o2 + 1) * 512], pb[:])

        # ---- expert loop ----
        n_chunks = (KH + KH_CHUNK - 1) // KH_CHUNK
        for n in range(N):
            # mm1: hT[m] = relu(W1T_n.T-tiles @ xt + b1)
            ht = []
            for m in range(MH):
                w1m = w1p.tile([P, KD, P], f32, tag="w1")
                nc.sync.dma_start(w1m[:], w1_d[n, m])
                ps1 = ps_1.tile([P, Bc], f32, tag="ps1")
                for k in range(KD):
                    nc.tensor.matmul(
                        ps1[:],
                        w1m[:, k, :],
                        xt[:, k, :],
                        start=(k == 0),
                        stop=(k == KD - 1),
                    )
                h = htp.tile([P, Bc], f32, tag="ht")
                nc.scalar.activation(
                    h[:], ps1[:], Relu, bias=b1_sb[:, n, m : m + 1]
                )
                ht.append(h)

            # mm2: acc[m][:, o2] += w_n * (hT.T @ W2T_n), chunked over kh
            for c in range(n_chunks):
                kh_lo = c * KH_CHUNK
                kh_hi = min(KH, kh_lo + KH_CHUNK)
                slabs = {}
                for kh2 in range(kh_lo // 2, (kh_hi + 1) // 2):
                    sl = w2p.tile([P, 2, O], f32, tag="w2")
                    nc.sync.dma_start(sl[:], w2_d[n, kh2])
                    slabs[kh2] = sl
                for m in range(MB):
                    for o2 in range(NO):
                        ps2 = ps_2.tile([P, 512], f32, tag="ps2")
                        for kh in range(kh_lo, kh_hi):
                            nc.tensor.matmul(
                                ps2[:],
                                ht[kh][:, m * P : (m + 1) * P],
                                slabs[kh // 2][:, kh % 2, o2 * 512 : (o2 + 1) * 512],
                                start=(kh == kh_lo),
                                stop=(kh == kh_hi - 1),
                            )
                        a = acc[m][:, o2 * 512 : (o2 + 1) * 512]
                        nc.vector.scalar_tensor_tensor(
                            a,
                            ps2[:],
                            w_sb[:, m, n : n + 1],
                            a,
                            op0=Alu.mult,
                            op1=Alu.add,
                        )

        # ---- store ----
        for m in range(MB):
            nc.sync.dma_start(out_d[m * P : (m + 1) * P, :], acc[m][:])

    nc.compile()
    return nc


def pack_inputs(x, W1, b1, W2, b2, Wg, bg, Bc, ncores):
    """Host-side shard + relayout. Returns per-core input maps."""
    P = 128
    N, H, D = W1.shape
    O = W2.shape[1]
    KD, MH, KH2 = D // P, H // P, H // P // 2

    x = np.ascontiguousarray(x, np.float32)
    # w1t[n, m, p, k, q] = W1[n, m*P+q, k*P+p]  (p = d partition, q = h free)
    w1t = np.ascontiguousarray(
        W1.reshape(N, MH, P, KD, P).transpose(0, 1, 4, 3, 2), np.float32
    )
    w2t = np.ascontiguousarray(
        W2.transpose(0, 2, 1).reshape(N, KH2, 2, P, O).transpose(0, 1, 3, 2, 4),
        np.float32,
    )  # [n, kh2, p, c, o] with value W2[n, o, (kh2*2+c)*P+p]
    b1p = np.ascontiguousarray(
        b1.reshape(N, MH, P).transpose(2, 0, 1), np.float32
    )  # [p, n, m]
    wgt = np.ascontiguousarray(
        Wg.reshape(N, KD, P).transpose(2, 1, 0), np.float32
    )  # [p, k, n]
    bgr = np.ascontiguousarray(np.tile(bg[None, :], (P, 1)), np.float32)
    b2s = np.ascontiguousarray(b2, np.float32)

    in_maps = []
    for c in range(ncores):
        xs = x[c * Bc : (c + 1) * Bc, :]  # [Bc, D]
        xts = np.ascontiguousarray(
            xs.T.reshape(KD, P, Bc).transpose(1, 0, 2), np.float32
        )  # [p, k, b]
        in_maps.append(
            {
                "xt": xts,
                "w1t": w1t,
                "w2t": w2t,
                "b1p": b1p,
                "b2s": b2s,
                "wgt": wgt,
                "bgr": bgr,
            }
        )
    return in_maps


_NC_CACHE = {}


def _get_nc():
    key = (B_FULL // NCORES, D_FULL, H_FULL, O_FULL)
    if key not in _NC_CACHE:
        _NC_CACHE[key] = build_moe_bass(
            B_FULL // NCORES, D_FULL, H_FULL, O_FULL, NEXP, TEMP
        )
    return _NC_CACHE[key]


def kernel(x, W1, b1, W2, b2, Wg, bg):
    from concourse.bass_utils import run_bass_kernel_spmd

    Bc = B_FULL // NCORES
    nc = _get_nc()
    in_maps = pack_inputs(
        np.asarray(x),
        np.asarray(W1),
        np.asarray(b1),
        np.asarray(W2),
        np.asarray(b2),
        np.asarray(Wg),
        np.asarray(bg),
        Bc,
        NCORES,
    )
    res = run_bass_kernel_spmd(nc, in_maps, core_ids=list(range(NCORES)))
    return np.concatenate([res.results[c]["out"] for c in range(NCORES)], axis=0)


# revision 12
# speedup vs baseline: 3.1810x; 3.1810x over previous
"""MoE (8 experts, top-5 Boltzmann gate) Trainium2 kernel.

Strategy: data-parallel over tokens. Each of the 8 NeuronCores processes
B/8 = 512 tokens and runs all 8 experts fused (hT stays in SBUF between
the two matmuls); gate weights are applied per-partition at PSUM evict.
No collectives — the host slices tokens and concatenates the outputs.

Host-side prep (sharding/layout): weights are pre-transposed and tiled so
every DMA is contiguous per partition.
"""

import numpy as np

# Problem dims (hardcoded per contract)
D_FULL, H_FULL, O_FULL, NEXP = 1024, 4096, 1024, 8
B_FULL = 4096
NCORES = 8
TEMP = float(np.e)
BIG = 1.0e30
KH_CHUNK = 8  # mm2 contraction tiles per PSUM accumulation group


def build_moe_bass(Bc, D, H, O, N, temp, num_devices=NCORES):
    """Build the per-core Bass/Tile program. Bc = tokens per core (<=512)."""
    from contextlib import ExitStack

    import concourse.bass as bass
    import concourse.tile as tile
    from concourse import bacc, mybir

    f32 = mybir.dt.float32
    # fp32-reduced (FP22 multiply, fp32 accumulate): full-rate PE streaming
    # (1 cycle/row for moving dim >= 256) vs 2-4x slower for plain fp32.
    f32r = mybir.dt.float32r
    P = 128
    assert Bc % P == 0 and Bc <= 512
    assert D % P == 0 and H % (2 * P) == 0 and O % 512 == 0
    KD, KH, MB, NO = D // P, H // P, Bc // P, O // 512
    MH = H // P
    KH2 = KH // 2  # kh pairs (w2 slab granularity)

    nc = bacc.Bacc(
        "TRN2", target_bir_lowering=False, debug=False, num_devices=num_devices
    )

    # DRAM I/O (host-packed layouts; all per-partition contiguous)
    xt_d = nc.dram_tensor("xt", [P, KD, Bc], f32r, kind="ExternalInput").ap()
    xg_d = nc.dram_tensor("xtg", [P, KD, Bc], f32, kind="ExternalInput").ap()
    w1_d = nc.dram_tensor("w1t", [N, MH, P, KD, P], f32r, kind="ExternalInput").ap()
    w2_d = nc.dram_tensor("w2t", [N, KH2, P, 2, O], f32r, kind="ExternalInput").ap()
    b1_d = nc.dram_tensor("b1p", [P, N, MH], f32, kind="ExternalInput").ap()
    b2_d = nc.dram_tensor("b2s", [N, O], f32, kind="ExternalInput").ap()
    wg_d = nc.dram_tensor("wgt", [P, KD, N], f32, kind="ExternalInput").ap()
    bg_d = nc.dram_tensor("bgr", [P, N], f32, kind="ExternalInput").ap()
    out_d = nc.dram_tensor("out", [Bc, O], f32, kind="ExternalOutput").ap()

    Exp = mybir.ActivationFunctionType.Exp
    Relu = mybir.ActivationFunctionType.Relu
    Alu = mybir.AluOpType

    with tile.TileContext(nc) as tc, ExitStack() as ctx:
        const = ctx.enter_context(tc.tile_pool(name="const", bufs=1))
        gatep = ctx.enter_context(tc.tile_pool(name="gate", bufs=2))
        xtp = ctx.enter_context(tc.tile_pool(name="xt", bufs=1))
        w1p = ctx.enter_context(tc.tile_pool(name="w1", bufs=3))
        w2p = ctx.enter_context(tc.tile_pool(name="w2", bufs=6))
        htp = ctx.enter_context(tc.tile_pool(name="ht", bufs=MH + 1))
        accp = ctx.enter_context(tc.tile_pool(name="acc", bufs=MB))
        ps_s = ctx.enter_context(tc.tile_pool(name="ps_s", bufs=2, space="PSUM"))
        ps_1 = ctx.enter_context(tc.tile_pool(name="ps_1", bufs=3, space="PSUM"))
        ps_2 = ctx.enter_context(tc.tile_pool(name="ps_2", bufs=3, space="PSUM"))

        # ---- constant / input loads ----
        xt = xtp.tile([P, KD, Bc], f32r)
        nc.sync.dma_start(xt[:], xt_d[:])
        xtg = xtp.tile([P, KD, Bc], f32, tag="xtg")
        nc.sync.dma_start(xtg[:], xg_d[:])
        wg_sb = const.tile([P, KD, N], f32)
        nc.sync.dma_start(wg_sb[:], wg_d[:])
        bg_sb = const.tile([P, N], f32)
        nc.sync.dma_start(bg_sb[:], bg_d[:])
        b1_sb = const.tile([P, N, MH], f32)
        nc.sync.dma_start(b1_sb[:], b1_d[:])
        b2_sb = const.tile([N, O], f32)
        nc.sync.dma_start(b2_sb[:], b2_d[:])

        w_sb = const.tile([P, MB, N], f32)  # gate weights, [token_p, mtile, expert]
        wt_sb = const.tile([32, Bc], f32)  # transposed gate weights (rows 0..N-1)

        # ---- gate: logits -> softmax(l/T) -> top-5 mask -> renormalize ----
        for m in range(MB):
            pg = ps_s.tile([P, N], f32, tag="ps_small")
            for k in range(KD):
                nc.tensor.matmul(
                    pg[:],
                    xtg[:, k, m * P : (m + 1) * P],
                    wg_sb[:, k, :],
                    start=(k == 0),
                    stop=(k == KD - 1),
                )
            lg = gatep.tile([P, N], f32, tag="g_l")
            nc.vector.tensor_tensor(lg[:], pg[:], bg_sb[:], Alu.add)
            rmax = gatep.tile([P, 1], f32, tag="g_max")
            nc.vector.reduce_max(rmax[:], lg[:], axis=mybir.AxisListType.X)
            nbias = gatep.tile([P, 1], f32, tag="g_nb")
            nc.scalar.mul(nbias[:], rmax[:], -1.0 / temp)
            e = gatep.tile([P, N], f32, tag="g_e")
            nc.scalar.activation(e[:], lg[:], Exp, bias=nbias[:], scale=1.0 / temp)
            z = gatep.tile([P, 1], f32, tag="g_z")
            nc.vector.reduce_sum(z[:], e[:], axis=mybir.AxisListType.X)
            zi = gatep.tile([P, 1], f32, tag="g_zi")
            nc.vector.reciprocal(zi[:], z[:])
            p = gatep.tile([P, N], f32, tag="g_p")
            nc.vector.tensor_scalar_mul(p[:], e[:], zi[:])
            # 3rd-smallest per row via iterated min-masking (drop bottom N-NA=3)
            cur = p
            mn = None
            for r in range(3):
                mn = gatep.tile([P, 1], f32, tag=f"g_mn{r}")
                nc.vector.tensor_reduce(
                    mn[:], cur[:], axis=mybir.AxisListType.X, op=Alu.min
                )
                if r < 2:
                    msk = gatep.tile([P, N], f32, tag=f"g_msk{r}")
                    nc.vector.tensor_scalar(
                        msk[:], cur[:], mn[:], BIG, op0=Alu.is_equal, op1=Alu.mult
                    )
                    nxt = gatep.tile([P, N], f32, tag=f"g_nxt{r}")
                    nc.vector.tensor_tensor(nxt[:], msk[:], cur[:], Alu.max)
                    cur = nxt
            pm = gatep.tile([P, N], f32, tag="g_pm")
            nc.vector.scalar_tensor_tensor(
                pm[:], p[:], mn[:], p[:], op0=Alu.is_gt, op1=Alu.mult
            )
            s = gatep.tile([P, 1], f32, tag="g_s")
            nc.vector.reduce_sum(s[:], pm[:], axis=mybir.AxisListType.X)
            se = gatep.tile([P, 1], f32, tag="g_se")
            nc.vector.tensor_scalar_add(se[:], s[:], 1.0e-8)
            si = gatep.tile([P, 1], f32, tag="g_si")
            nc.vector.reciprocal(si[:], se[:])
            nc.vector.tensor_scalar_mul(w_sb[:, m, :], pm[:], si[:])

            # transpose this m-tile's gate weights into wt_sb[0:N, m*P:(m+1)*P]
            wpad = gatep.tile([P, 32], f32, tag="g_wpad")
            nc.vector.memset(wpad[:], 0.0)
            nc.vector.tensor_copy(wpad[:, 0:N], w_sb[:, m, :])
            for blk in range(4):
                nc.vector.transpose(
                    wt_sb[0:32, m * P + 32 * blk : m * P + 32 * (blk + 1)],
                    wpad[32 * blk : 32 * (blk + 1), 0:32],
                )

        # ---- out_acc init: b2 contribution = w @ b2_stack (K = N experts) ----
        acc = [
            accp.tile([P, O], f32, name=f"acc{m}", tag="acc") for m in range(MB)
        ]
        for m in range(MB):
            for o2 in range(NO):
                pb = ps_s.tile([P, 512], f32, tag="ps_small")
                nc.tensor.matmul(
                    pb[:],
                    wt_sb[0:N, m * P : (m + 1) * P],
                    b2_sb[0:N, o2 * 512 : (o2 + 1) * 512],
                    start=True,
                    stop=True,
                )
                nc.vector.tensor_copy(acc[m][:, o2 * 512 : (o2 + 1) * 512], pb[:])

        # ---- expert loop ----
        n_chunks = (KH + KH_CHUNK - 1) // KH_CHUNK
        for n in range(N):
            # mm1: hT[m] = relu(W1T_n.T-tiles @ xt + b1)
            ht = []
            for m in range(MH):
                w1m = w1p.tile([P, KD, P], f32r, tag="w1")
                nc.sync.dma_start(w1m[:], w1_d[n, m])
                ps1 = ps_1.tile([P, Bc], f32, tag="ps1")
                for k in range(KD):
                    nc.tensor.matmul(
                        ps1[:],
                        w1m[:, k, :],
                        xt[:, k, :],
                        start=(k == 0),
                        stop=(k == KD - 1),
                    )
                h = htp.tile([P, Bc], f32r, tag="ht")
                nc.scalar.activation(
                    h[:], ps1[:], Relu, bias=b1_sb[:, n, m : m + 1]
                )
                ht.append(h)

            # mm2: acc[m][:, o2] += w_n * (hT.T @ W2T_n), chunked over kh
            for c in range(n_chunks):
                kh_lo = c * KH_CHUNK
                kh_hi = min(KH, kh_lo + KH_CHUNK)
                slabs = {}
                for kh2 in range(kh_lo // 2, (kh_hi + 1) // 2):
                    sl = w2p.tile([P, 2, O], f32r, tag="w2")
                    nc.sync.dma_start(sl[:], w2_d[n, kh2])
                    slabs[kh2] = sl
                for m in range(MB):
                    for o2 in range(NO):
                        ps2 = ps_2.tile([P, 512], f32, tag="ps2")
                        for kh in range(kh_lo, kh_hi):
                            nc.tensor.matmul(
                                ps2[:],
                                ht[kh][:, m * P : (m + 1) * P],
                                slabs[kh // 2][:, kh % 2, o2 * 512 : (o2 + 1) * 512],
                                start=(kh == kh_lo),
                                stop=(kh == kh_hi - 1),
                            )
                        a = acc[m][:, o2 * 512 : (o2 + 1) * 512]
                        nc.vector.scalar_tensor_tensor(
                            a,
                            ps2[:],
                            w_sb[:, m, n : n + 1],
                            a,
                            op0=Alu.mult,
                            op1=Alu.add,
                        )

        # ---- store ----
        for m in range(MB):
            nc.sync.dma_start(out_d[m * P : (m + 1) * P, :], acc[m][:])

    nc.compile()
    return nc


def pack_inputs(x, W1, b1, W2, b2, Wg, bg, Bc, ncores):
    """Host-side shard + relayout. Returns per-core input maps."""
    P = 128
    N, H, D = W1.shape
    O = W2.shape[1]
    KD, MH, KH2 = D // P, H // P, H // P // 2

    x = np.ascontiguousarray(x, np.float32)
    # w1t[n, m, p, k, q] = W1[n, m*P+q, k*P+p]  (p = d partition, q = h free)
    w1t = np.ascontiguousarray(
        W1.reshape(N, MH, P, KD, P).transpose(0, 1, 4, 3, 2), np.float32
    )
    w2t = np.ascontiguousarray(
        W2.transpose(0, 2, 1).reshape(N, KH2, 2, P, O).transpose(0, 1, 3, 2, 4),
        np.float32,
    )  # [n, kh2, p, c, o] with value W2[n, o, (kh2*2+c)*P+p]
    b1p = np.ascontiguousarray(
        b1.reshape(N, MH, P).transpose(2, 0, 1), np.float32
    )  # [p, n, m]
    wgt = np.ascontiguousarray(
        Wg.reshape(N, KD, P).transpose(2, 1, 0), np.float32
    )  # [p, k, n]
    bgr = np.ascontiguousarray(np.tile(bg[None, :], (P, 1)), np.float32)
    b2s = np.ascontiguousarray(b2, np.float32)

    in_maps = []
    for c in range(ncores):
        xs = x[c * Bc : (c + 1) * Bc, :]  # [Bc, D]
        xts = np.ascontiguousarray(
            xs.T.reshape(KD, P, Bc).transpose(1, 0, 2), np.float32
        )  # [p, k, b]
        in_maps.append(
            {
                "xt": xts,
                "xtg": xts,
                "w1t": w1t,
                "w2t": w2t,
                "b1p": b1p,
                "b2s": b2s,
                "wgt": wgt,
                "bgr": bgr,
            }
        )
    return in_maps


_NC_CACHE = {}


def _get_nc():
    key = (B_FULL // NCORES, D_FULL, H_FULL, O_FULL)
    if key not in _NC_CACHE:
        _NC_CACHE[key] = build_moe_bass(
            B_FULL // NCORES, D_FULL, H_FULL, O_FULL, NEXP, TEMP
        )
    return _NC_CACHE[key]


def kernel(x, W1, b1, W2, b2, Wg, bg):
    from concourse.bass_utils import run_bass_kernel_spmd

    Bc = B_FULL // NCORES
    nc = _get_nc()
    in_maps = pack_inputs(
        np.asarray(x),
        np.asarray(W1),
        np.asarray(b1),
        np.asarray(W2),
        np.asarray(b2),
        np.asarray(Wg),
        np.asarray(bg),
        Bc,
        NCORES,
    )
    res = run_bass_kernel_spmd(nc, in_maps, core_ids=list(range(NCORES)))
    return np.concatenate([res.results[c]["out"] for c in range(NCORES)], axis=0)


# revision 22
# speedup vs baseline: 3.6415x; 1.1448x over previous
"""MoE (8 experts, top-5 Boltzmann gate) Trainium2 kernel.

Strategy: data-parallel over tokens. Each of the 8 NeuronCores processes
B/8 = 512 tokens and runs all 8 experts fused (hT stays in SBUF between
the two matmuls); gate weights are applied per-partition at PSUM evict.
No collectives — the host slices tokens and concatenates the outputs.

Host-side prep (sharding/layout): weights are pre-transposed and tiled so
every DMA is contiguous per partition.
"""

import numpy as np

# Problem dims (hardcoded per contract)
D_FULL, H_FULL, O_FULL, NEXP = 1024, 4096, 1024, 8
B_FULL = 4096
NCORES = 8
TEMP = float(np.e)
BIG = 1.0e30
KH_CHUNK = 8  # mm2 contraction tiles per PSUM accumulation group


def build_moe_bass(Bc, D, H, O, N, temp, num_devices=NCORES):
    """Build the per-core Bass/Tile program. Bc = tokens per core (<=512)."""
    from contextlib import ExitStack

    import concourse.bass as bass
    import concourse.tile as tile
    from concourse import bacc, mybir

    f32 = mybir.dt.float32
    # fp16 operands for the heavy matmuls: full-rate PE streaming, FWL weight
    # loads, and half the HBM traffic. PSUM accumulation stays fp32.
    f16 = mybir.dt.float16
    P = 128
    assert Bc % P == 0 and Bc <= 512
    assert D % P == 0 and H % (2 * P) == 0 and O % 512 == 0
    KD, KH, MB, NO = D // P, H // P, Bc // P, O // 512
    MH = H // P
    KH2 = KH // 2  # kh pairs (w2 slab granularity)

    nc = bacc.Bacc(
        "TRN2", target_bir_lowering=False, debug=False, num_devices=num_devices
    )

    # DRAM I/O (host-packed layouts; all per-partition contiguous)
    xt_d = nc.dram_tensor("xt", [P, KD, Bc], f16, kind="ExternalInput").ap()
    xg_d = nc.dram_tensor("xtg", [P, KD, Bc], f32, kind="ExternalInput").ap()
    w1_d = nc.dram_tensor("w1t", [N, MH, P, KD, P], f16, kind="ExternalInput").ap()
    w2_d = nc.dram_tensor("w2t", [N, KH2, P, 2, O], f16, kind="ExternalInput").ap()
    b1_d = nc.dram_tensor("b1p", [P, N, MH], f32, kind="ExternalInput").ap()
    b2_d = nc.dram_tensor("b2s", [N, O], f32, kind="ExternalInput").ap()
    wg_d = nc.dram_tensor("wgt", [P, KD, N], f32, kind="ExternalInput").ap()
    bg_d = nc.dram_tensor("bgr", [P, N], f32, kind="ExternalInput").ap()
    out_d = nc.dram_tensor("out", [Bc, O], f32, kind="ExternalOutput").ap()

    Exp = mybir.ActivationFunctionType.Exp
    Relu = mybir.ActivationFunctionType.Relu
    Alu = mybir.AluOpType

    with tile.TileContext(nc) as tc, ExitStack() as ctx:
        const = ctx.enter_context(tc.tile_pool(name="const", bufs=1))
        gatep = ctx.enter_context(tc.tile_pool(name="gate", bufs=2))
        xtp = ctx.enter_context(tc.tile_pool(name="xt", bufs=1))
        w1p = ctx.enter_context(tc.tile_pool(name="w1", bufs=3))
        w2p = ctx.enter_context(tc.tile_pool(name="w2", bufs=6))
        htp = ctx.enter_context(tc.tile_pool(name="ht", bufs=MH + 1))
        accp = ctx.enter_context(tc.tile_pool(name="acc", bufs=MB))
        ps_s = ctx.enter_context(tc.tile_pool(name="ps_s", bufs=2, space="PSUM"))
        ps_1 = ctx.enter_context(tc.tile_pool(name="ps_1", bufs=3, space="PSUM"))
        ps_2 = ctx.enter_context(tc.tile_pool(name="ps_2", bufs=3, space="PSUM"))

        # ---- constant / input loads ----
        xt = xtp.tile([P, KD, Bc], f16)
        nc.sync.dma_start(xt[:], xt_d[:])
        xtg = xtp.tile([P, KD, Bc], f32, tag="xtg")
        nc.sync.dma_start(xtg[:], xg_d[:])
        wg_sb = const.tile([P, KD, N], f32)
        nc.sync.dma_start(wg_sb[:], wg_d[:])
        bg_sb = const.tile([P, N], f32)
        nc.sync.dma_start(bg_sb[:], bg_d[:])
        b1_sb = const.tile([P, N, MH], f32)
        nc.sync.dma_start(b1_sb[:], b1_d[:])
        b2_sb = const.tile([N, O], f32)
        nc.sync.dma_start(b2_sb[:], b2_d[:])

        w_sb = const.tile([P, MB, N], f32)  # gate weights, [token_p, mtile, expert]
        wt_sb = const.tile([32, Bc], f32)  # transposed gate weights (rows 0..N-1)

        def emit_mm1(n):
            """hT[m] = relu(W1T_n-tiles.T @ xt + b1) for all H tiles; fp16 out."""
            ht = []
            for m in range(MH):
                w1m = w1p.tile([P, KD, P], f16, tag="w1", name=f"w1m_{n}_{m}")
                nc.sync.dma_start(w1m[:], w1_d[n, m])
                ps1 = ps_1.tile([P, Bc], f32, tag="ps1", name=f"ps1_{n}_{m}")
                for k in range(KD):
                    nc.tensor.matmul(
                        ps1[:],
                        w1m[:, k, :],
                        xt[:, k, :],
                        start=(k == 0),
                        stop=(k == KD - 1),
                    )
                h = htp.tile([P, Bc], f16, tag="ht", name=f"ht_{n}_{m}")
                nc.scalar.activation(h[:], ps1[:], Relu, bias=b1_sb[:, n, m : m + 1])
                ht.append(h)
            return ht

        # expert 0 mm1 first so the PE starts as soon as xt + first w1 land
        ht0 = emit_mm1(0)

        # ---- gate: logits -> softmax(l/T) -> top-5 mask -> renormalize ----
        for m in range(MB):
            pg = ps_s.tile([P, N], f32, tag="ps_small")
            for k in range(KD):
                nc.tensor.matmul(
                    pg[:],
                    xtg[:, k, m * P : (m + 1) * P],
                    wg_sb[:, k, :],
                    start=(k == 0),
                    stop=(k == KD - 1),
                )
            lg = gatep.tile([P, N], f32, tag="g_l")
            nc.vector.tensor_tensor(lg[:], pg[:], bg_sb[:], Alu.add)
            rmax = gatep.tile([P, 1], f32, tag="g_max")
            nc.vector.reduce_max(rmax[:], lg[:], axis=mybir.AxisListType.X)
            nbias = gatep.tile([P, 1], f32, tag="g_nb")
            nc.scalar.mul(nbias[:], rmax[:], -1.0 / temp)
            e = gatep.tile([P, N], f32, tag="g_e")
            nc.scalar.activation(e[:], lg[:], Exp, bias=nbias[:], scale=1.0 / temp)
            z = gatep.tile([P, 1], f32, tag="g_z")
            nc.vector.reduce_sum(z[:], e[:], axis=mybir.AxisListType.X)
            zi = gatep.tile([P, 1], f32, tag="g_zi")
            nc.vector.reciprocal(zi[:], z[:])
            p = gatep.tile([P, N], f32, tag="g_p")
            nc.vector.tensor_scalar_mul(p[:], e[:], zi[:])
            # 3rd-smallest per row via iterated min-masking (drop bottom N-NA=3)
            cur = p
            mn = None
            for r in range(3):
                mn = gatep.tile([P, 1], f32, tag=f"g_mn{r}")
                nc.vector.tensor_reduce(
                    mn[:], cur[:], axis=mybir.AxisListType.X, op=Alu.min
                )
                if r < 2:
                    msk = gatep.tile([P, N], f32, tag=f"g_msk{r}")
                    nc.vector.tensor_scalar(
                        msk[:], cur[:], mn[:], BIG, op0=Alu.is_equal, op1=Alu.mult
                    )
                    nxt = gatep.tile([P, N], f32, tag=f"g_nxt{r}")
                    nc.vector.tensor_tensor(nxt[:], msk[:], cur[:], Alu.max)
                    cur = nxt
            pm = gatep.tile([P, N], f32, tag="g_pm")
            nc.vector.scalar_tensor_tensor(
                pm[:], p[:], mn[:], p[:], op0=Alu.is_gt, op1=Alu.mult
            )
            s = gatep.tile([P, 1], f32, tag="g_s")
            nc.vector.reduce_sum(s[:], pm[:], axis=mybir.AxisListType.X)
            se = gatep.tile([P, 1], f32, tag="g_se")
            nc.vector.tensor_scalar_add(se[:], s[:], 1.0e-8)
            si = gatep.tile([P, 1], f32, tag="g_si")
            nc.vector.reciprocal(si[:], se[:])
            nc.vector.tensor_scalar_mul(w_sb[:, m, :], pm[:], si[:])

            # transpose this m-tile's gate weights into wt_sb[0:N, m*P:(m+1)*P]
            wpad = gatep.tile([P, 32], f32, tag="g_wpad")
            nc.vector.memset(wpad[:], 0.0)
            nc.vector.tensor_copy(wpad[:, 0:N], w_sb[:, m, :])
            for blk in range(4):
                nc.vector.transpose(
                    wt_sb[0:32, m * P + 32 * blk : m * P + 32 * (blk + 1)],
                    wpad[32 * blk : 32 * (blk + 1), 0:32],
                )

        # ---- out_acc init: b2 contribution = w @ b2_stack (K = N experts) ----
        acc = [
            accp.tile([P, O], f32, name=f"acc{m}", tag="acc") for m in range(MB)
        ]
        for m in range(MB):
            for o2 in range(NO):
                pb = ps_s.tile([P, 512], f32, tag="ps_small")
                nc.tensor.matmul(
                    pb[:],
                    wt_sb[0:N, m * P : (m + 1) * P],
                    b2_sb[0:N, o2 * 512 : (o2 + 1) * 512],
                    start=True,
                    stop=True,
                )
                nc.vector.tensor_copy(acc[m][:, o2 * 512 : (o2 + 1) * 512], pb[:])

        # ---- expert loop ----
        n_chunks = (KH + KH_CHUNK - 1) // KH_CHUNK

        def emit_mm2(n, ht):
            """acc[m][:, o2] += w_n * (hT.T @ W2T_n), chunked over kh."""
            for c in range(n_chunks):
                kh_lo = c * KH_CHUNK
                kh_hi = min(KH, kh_lo + KH_CHUNK)
                slabs = {}
                for kh2 in range(kh_lo // 2, (kh_hi + 1) // 2):
                    sl = w2p.tile([P, 2, O], f16, tag="w2", name=f"w2_{n}_{kh2}")
                    nc.sync.dma_start(sl[:], w2_d[n, kh2])
                    slabs[kh2] = sl
                for m in range(MB):
                    for o2 in range(NO):
                        ps2 = ps_2.tile(
                            [P, 512], f32, tag="ps2", name=f"ps2_{n}_{c}_{m}_{o2}"
                        )
                        for kh in range(kh_lo, kh_hi):
                            nc.tensor.matmul(
                                ps2[:],
                                ht[kh][:, m * P : (m + 1) * P],
                                slabs[kh // 2][:, kh % 2, o2 * 512 : (o2 + 1) * 512],
                                start=(kh == kh_lo),
                                stop=(kh == kh_hi - 1),
                            )
                        a = acc[m][:, o2 * 512 : (o2 + 1) * 512]
                        nc.vector.scalar_tensor_tensor(
                            a,
                            ps2[:],
                            w_sb[:, m, n : n + 1],
                            a,
                            op0=Alu.mult,
                            op1=Alu.add,
                        )

        emit_mm2(0, ht0)
        for n in range(1, N):
            ht = emit_mm1(n)
            emit_mm2(n, ht)

        # ---- store ----
        for m in range(MB):
            nc.sync.dma_start(out_d[m * P : (m + 1) * P, :], acc[m][:])

    nc.compile()
    return nc


def pack_inputs(x, W1, b1, W2, b2, Wg, bg, Bc, ncores):
    """Host-side shard + relayout. Returns per-core input maps."""
    P = 128
    N, H, D = W1.shape
    O = W2.shape[1]
    KD, MH, KH2 = D // P, H // P, H // P // 2

    x = np.ascontiguousarray(x, np.float32)
    # w1t[n, m, p, k, q] = W1[n, m*P+q, k*P+p]  (p = d partition, q = h free)
    w1t = np.ascontiguousarray(
        W1.reshape(N, MH, P, KD, P).transpose(0, 1, 4, 3, 2), np.float16
    )
    w2t = np.ascontiguousarray(
        W2.transpose(0, 2, 1).reshape(N, KH2, 2, P, O).transpose(0, 1, 3, 2, 4),
        np.float16,
    )  # [n, kh2, p, c, o] with value W2[n, o, (kh2*2+c)*P+p]
    b1p = np.ascontiguousarray(
        b1.reshape(N, MH, P).transpose(2, 0, 1), np.float32
    )  # [p, n, m]
    wgt = np.ascontiguousarray(
        Wg.reshape(N, KD, P).transpose(2, 1, 0), np.float32
    )  # [p, k, n]
    bgr = np.ascontiguousarray(np.tile(bg[None, :], (P, 1)), np.float32)
    b2s = np.ascontiguousarray(b2, np.float32)

    in_maps = []
    for c in range(ncores):
        xs = x[c * Bc : (c + 1) * Bc, :]  # [Bc, D]
        xts = np.ascontiguousarray(
            xs.T.reshape(KD, P, Bc).transpose(1, 0, 2), np.float32
        )  # [p, k, b]
        in_maps.append(
            {
                "xt": xts.astype(np.float16),
                "xtg": xts,
                "w1t": w1t,
                "w2t": w2t,
                "b1p": b1p,
                "b2s": b2s,
                "wgt": wgt,
                "bgr": bgr,
            }
        )
    return in_maps


_NC_CACHE = {}


def _get_nc():
    key = (B_FULL // NCORES, D_FULL, H_FULL, O_FULL)
    if key not in _NC_CACHE:
        _NC_CACHE[key] = build_moe_bass(
            B_FULL // NCORES, D_FULL, H_FULL, O_FULL, NEXP, TEMP
        )
    return _NC_CACHE[key]


def kernel(x, W1, b1, W2, b2, Wg, bg):
    from concourse.bass_utils import run_bass_kernel_spmd

    Bc = B_FULL // NCORES
    nc = _get_nc()
    in_maps = pack_inputs(
        np.asarray(x),
        np.asarray(W1),
        np.asarray(b1),
        np.asarray(W2),
        np.asarray(b2),
        np.asarray(Wg),
        np.asarray(bg),
        Bc,
        NCORES,
    )
    res = run_bass_kernel_spmd(nc, in_maps, core_ids=list(range(NCORES)))
    return np.concatenate([res.results[c]["out"] for c in range(NCORES)], axis=0)


# revision 26
# speedup vs baseline: 3.7408x; 1.0272x over previous
"""MoE (8 experts, top-5 Boltzmann gate) Trainium2 kernel.

Strategy: data-parallel over tokens. Each of the 8 NeuronCores processes
B/8 = 512 tokens and runs all 8 experts fused (hT stays in SBUF between
the two matmuls); gate weights are applied per-partition at PSUM evict.
No collectives — the host slices tokens and concatenates the outputs.

Host-side prep (sharding/layout): weights are pre-transposed and tiled so
every DMA is contiguous per partition.
"""

import numpy as np

# Problem dims (hardcoded per contract)
D_FULL, H_FULL, O_FULL, NEXP = 1024, 4096, 1024, 8
B_FULL = 4096
NCORES = 8
TEMP = float(np.e)
BIG = 1.0e30
KH_CHUNK = 16  # mm2 contraction tiles per PSUM accumulation group


def build_moe_bass(Bc, D, H, O, N, temp, num_devices=NCORES):
    """Build the per-core Bass/Tile program. Bc = tokens per core (<=512)."""
    from contextlib import ExitStack

    import concourse.bass as bass
    import concourse.tile as tile
    from concourse import bacc, mybir

    f32 = mybir.dt.float32
    # fp16 operands for the heavy matmuls: full-rate PE streaming, FWL weight
    # loads, and half the HBM traffic. PSUM accumulation stays fp32.
    f16 = mybir.dt.float16
    P = 128
    assert Bc % P == 0 and Bc <= 512
    assert D % P == 0 and H % (2 * P) == 0 and O % 512 == 0
    KD, KH, MB, NO = D // P, H // P, Bc // P, O // 512
    MH = H // P
    KH2 = KH // 2  # kh pairs (w2 slab granularity)

    nc = bacc.Bacc(
        "TRN2", target_bir_lowering=False, debug=False, num_devices=num_devices
    )

    # DRAM I/O (host-packed layouts; all per-partition contiguous)
    xt_d = nc.dram_tensor("xt", [P, KD, Bc], f16, kind="ExternalInput").ap()
    xg_d = nc.dram_tensor("xtg", [P, KD, Bc], f32, kind="ExternalInput").ap()
    w1_d = nc.dram_tensor("w1t", [N, MH, P, KD, P], f16, kind="ExternalInput").ap()
    w2_d = nc.dram_tensor("w2t", [N, KH2, P, 2, O], f16, kind="ExternalInput").ap()
    b1_d = nc.dram_tensor("b1p", [P, N, MH], f32, kind="ExternalInput").ap()
    b2_d = nc.dram_tensor("b2s", [N, O], f32, kind="ExternalInput").ap()
    wg_d = nc.dram_tensor("wgt", [P, KD, N], f32, kind="ExternalInput").ap()
    bg_d = nc.dram_tensor("bgr", [P, N], f32, kind="ExternalInput").ap()
    out_d = nc.dram_tensor("out", [Bc, O], f32, kind="ExternalOutput").ap()

    Exp = mybir.ActivationFunctionType.Exp
    Relu = mybir.ActivationFunctionType.Relu
    Alu = mybir.AluOpType

    with tile.TileContext(nc) as tc, ExitStack() as ctx:
        const = ctx.enter_context(tc.tile_pool(name="const", bufs=1))
        gatep = ctx.enter_context(tc.tile_pool(name="gate", bufs=2))
        xtp = ctx.enter_context(tc.tile_pool(name="xt", bufs=1))
        w1p = ctx.enter_context(tc.tile_pool(name="w1", bufs=6))
        w2p = ctx.enter_context(tc.tile_pool(name="w2", bufs=12))
        htp = ctx.enter_context(tc.tile_pool(name="ht", bufs=MH + 1))
        accp = ctx.enter_context(tc.tile_pool(name="acc", bufs=MB))
        ps_s = ctx.enter_context(tc.tile_pool(name="ps_s", bufs=2, space="PSUM"))
        ps_1 = ctx.enter_context(tc.tile_pool(name="ps_1", bufs=3, space="PSUM"))
        ps_2 = ctx.enter_context(tc.tile_pool(name="ps_2", bufs=3, space="PSUM"))

        # ---- constant / input loads ----
        # Two HWDGE rings (SP='sync', ACT='scalar'), each FIFO: keep xt + w1
        # on the sync ring so mm1 starts ASAP; consts/xtg/w2 go via scalar.
        xt = xtp.tile([P, KD, Bc], f16)
        nc.sync.dma_start(xt[:], xt_d[:])
        xtg = xtp.tile([P, KD, Bc], f32, tag="xtg")
        nc.scalar.dma_start(xtg[:], xg_d[:])
        wg_sb = const.tile([P, KD, N], f32)
        nc.scalar.dma_start(wg_sb[:], wg_d[:])
        bg_sb = const.tile([P, N], f32)
        nc.scalar.dma_start(bg_sb[:], bg_d[:])
        b1_sb = const.tile([P, N, MH], f32)
        nc.scalar.dma_start(b1_sb[:], b1_d[:])
        b2_sb = const.tile([N, O], f32)
        nc.scalar.dma_start(b2_sb[:], b2_d[:])

        w_sb = const.tile([P, MB, N], f32)  # gate weights, [token_p, mtile, expert]
        wt_sb = const.tile([32, Bc], f32)  # transposed gate weights (rows 0..N-1)

        def emit_mm1(n):
            """hT[m] = relu(W1T_n-tiles.T @ xt + b1) for all H tiles; fp16 out."""
            ht = []
            for m in range(MH):
                w1m = w1p.tile([P, KD, P], f16, tag="w1", name=f"w1m_{n}_{m}")
                nc.sync.dma_start(w1m[:], w1_d[n, m])
                ps1 = ps_1.tile([P, Bc], f32, tag="ps1", name=f"ps1_{n}_{m}")
                for k in range(KD):
                    nc.tensor.matmul(
                        ps1[:],
                        w1m[:, k, :],
                        xt[:, k, :],
                        start=(k == 0),
                        stop=(k == KD - 1),
                    )
                h = htp.tile([P, Bc], f16, tag="ht", name=f"ht_{n}_{m}")
                nc.scalar.activation(h[:], ps1[:], Relu, bias=b1_sb[:, n, m : m + 1])
                ht.append(h)
            return ht

        # expert 0 mm1 first so the PE starts as soon as xt + first w1 land
        ht0 = emit_mm1(0)

        # ---- gate: logits -> softmax(l/T) -> top-5 mask -> renormalize ----
        for m in range(MB):
            pg = ps_s.tile([P, N], f32, tag="ps_small")
            for k in range(KD):
                nc.tensor.matmul(
                    pg[:],
                    xtg[:, k, m * P : (m + 1) * P],
                    wg_sb[:, k, :],
                    start=(k == 0),
                    stop=(k == KD - 1),
                )
            lg = gatep.tile([P, N], f32, tag="g_l")
            nc.vector.tensor_tensor(lg[:], pg[:], bg_sb[:], Alu.add)
            rmax = gatep.tile([P, 1], f32, tag="g_max")
            nc.vector.reduce_max(rmax[:], lg[:], axis=mybir.AxisListType.X)
            nbias = gatep.tile([P, 1], f32, tag="g_nb")
            nc.scalar.mul(nbias[:], rmax[:], -1.0 / temp)
            e = gatep.tile([P, N], f32, tag="g_e")
            nc.scalar.activation(e[:], lg[:], Exp, bias=nbias[:], scale=1.0 / temp)
            z = gatep.tile([P, 1], f32, tag="g_z")
            nc.vector.reduce_sum(z[:], e[:], axis=mybir.AxisListType.X)
            zi = gatep.tile([P, 1], f32, tag="g_zi")
            nc.vector.reciprocal(zi[:], z[:])
            p = gatep.tile([P, N], f32, tag="g_p")
            nc.vector.tensor_scalar_mul(p[:], e[:], zi[:])
            # 3rd-smallest per row via iterated min-masking (drop bottom N-NA=3)
            cur = p
            mn = None
            for r in range(3):
                mn = gatep.tile([P, 1], f32, tag=f"g_mn{r}")
                nc.vector.tensor_reduce(
                    mn[:], cur[:], axis=mybir.AxisListType.X, op=Alu.min
                )
                if r < 2:
                    msk = gatep.tile([P, N], f32, tag=f"g_msk{r}")
                    nc.vector.tensor_scalar(
                        msk[:], cur[:], mn[:], BIG, op0=Alu.is_equal, op1=Alu.mult
                    )
                    nxt = gatep.tile([P, N], f32, tag=f"g_nxt{r}")
                    nc.vector.tensor_tensor(nxt[:], msk[:], cur[:], Alu.max)
                    cur = nxt
            pm = gatep.tile([P, N], f32, tag="g_pm")
            nc.vector.scalar_tensor_tensor(
                pm[:], p[:], mn[:], p[:], op0=Alu.is_gt, op1=Alu.mult
            )
            s = gatep.tile([P, 1], f32, tag="g_s")
            nc.vector.reduce_sum(s[:], pm[:], axis=mybir.AxisListType.X)
            se = gatep.tile([P, 1], f32, tag="g_se")
            nc.vector.tensor_scalar_add(se[:], s[:], 1.0e-8)
            si = gatep.tile([P, 1], f32, tag="g_si")
            nc.vector.reciprocal(si[:], se[:])
            nc.vector.tensor_scalar_mul(w_sb[:, m, :], pm[:], si[:])

            # transpose this m-tile's gate weights into wt_sb[0:N, m*P:(m+1)*P]
            wpad = gatep.tile([P, 32], f32, tag="g_wpad")
            nc.vector.memset(wpad[:], 0.0)
            nc.vector.tensor_copy(wpad[:, 0:N], w_sb[:, m, :])
            for blk in range(4):
                nc.vector.transpose(
                    wt_sb[0:32, m * P + 32 * blk : m * P + 32 * (blk + 1)],
                    wpad[32 * blk : 32 * (blk + 1), 0:32],
                )

        # ---- out_acc init: b2 contribution = w @ b2_stack (K = N experts) ----
        acc = [
            accp.tile([P, O], f32, name=f"acc{m}", tag="acc") for m in range(MB)
        ]
        for m in range(MB):
            for o2 in range(NO):
                pb = ps_s.tile([P, 512], f32, tag="ps_small")
                nc.tensor.matmul(
                    pb[:],
                    wt_sb[0:N, m * P : (m + 1) * P],
                    b2_sb[0:N, o2 * 512 : (o2 + 1) * 512],
                    start=True,
                    stop=True,
                )
                nc.vector.tensor_copy(acc[m][:, o2 * 512 : (o2 + 1) * 512], pb[:])

        # ---- expert loop ----
        n_chunks = (KH + KH_CHUNK - 1) // KH_CHUNK

        def emit_mm2(n, ht):
            """acc[m][:, o2] += w_n * (hT.T @ W2T_n), chunked over kh."""
            for c in range(n_chunks):
                kh_lo = c * KH_CHUNK
                kh_hi = min(KH, kh_lo + KH_CHUNK)
                slabs = {}
                for kh2 in range(kh_lo // 2, (kh_hi + 1) // 2):
                    sl = w2p.tile([P, 2, O], f16, tag="w2", name=f"w2_{n}_{kh2}")
                    nc.scalar.dma_start(sl[:], w2_d[n, kh2])
                    slabs[kh2] = sl
                for m in range(MB):
                    for o2 in range(NO):
                        ps2 = ps_2.tile(
                            [P, 512], f32, tag="ps2", name=f"ps2_{n}_{c}_{m}_{o2}"
                        )
                        for kh in range(kh_lo, kh_hi):
                            nc.tensor.matmul(
                                ps2[:],
                                ht[kh][:, m * P : (m + 1) * P],
                                slabs[kh // 2][:, kh % 2, o2 * 512 : (o2 + 1) * 512],
                                start=(kh == kh_lo),
                                stop=(kh == kh_hi - 1),
                            )
                        a = acc[m][:, o2 * 512 : (o2 + 1) * 512]
                        nc.vector.scalar_tensor_tensor(
                            a,
                            ps2[:],
                            w_sb[:, m, n : n + 1],
                            a,
                            op0=Alu.mult,
                            op1=Alu.add,
                        )

        emit_mm2(0, ht0)
        for n in range(1, N):
            ht = emit_mm1(n)
            emit_mm2(n, ht)

        # ---- store ----
        for m in range(MB):
            nc.sync.dma_start(out_d[m * P : (m + 1) * P, :], acc[m][:])

    nc.compile()
    return nc


def pack_inputs(x, W1, b1, W2, b2, Wg, bg, Bc, ncores):
    """Host-side shard + relayout. Returns per-core input maps."""
    P = 128
    N, H, D = W1.shape
    O = W2.shape[1]
    KD, MH, KH2 = D // P, H // P, H // P // 2

    x = np.ascontiguousarray(x, np.float32)
    # w1t[n, m, p, k, q] = W1[n, m*P+q, k*P+p]  (p = d partition, q = h free)
    w1t = np.ascontiguousarray(
        W1.reshape(N, MH, P, KD, P).transpose(0, 1, 4, 3, 2), np.float16
    )
    w2t = np.ascontiguousarray(
        W2.transpose(0, 2, 1).reshape(N, KH2, 2, P, O).transpose(0, 1, 3, 2, 4),
        np.float16,
    )  # [n, kh2, p, c, o] with value W2[n, o, (kh2*2+c)*P+p]
    b1p = np.ascontiguousarray(
        b1.reshape(N, MH, P).transpose(2, 0, 1), np.float32
    )  # [p, n, m]
    wgt = np.ascontiguousarray(
        Wg.reshape(N, KD, P).transpose(2, 1, 0), np.float32
    )  # [p, k, n]
    bgr = np.ascontiguousarray(np.tile(bg[None, :], (P, 1)), np.float32)
    b2s = np.ascontiguousarray(b2, np.float32)

    in_maps = []
    for c in range(ncores):
        xs = x[c * Bc : (c + 1) * Bc, :]  # [Bc, D]
        xts = np.ascontiguousarray(
            xs.T.reshape(KD, P, Bc).transpose(1, 0, 2), np.float32
        )  # [p, k, b]
        in_maps.append(
            {
                "xt": xts.astype(np.float16),
                "xtg": xts,
                "w1t": w1t,
                "w2t": w2t,
                "b1p": b1p,
                "b2s": b2s,
                "wgt": wgt,
                "bgr": bgr,
            }
        )
    return in_maps


_NC_CACHE = {}


def _get_nc():
    key = (B_FULL // NCORES, D_FULL, H_FULL, O_FULL)
    if key not in _NC_CACHE:
        _NC_CACHE[key] = build_moe_bass(
            B_FULL // NCORES, D_FULL, H_FULL, O_FULL, NEXP, TEMP
        )
    return _NC_CACHE[key]


def kernel(x, W1, b1, W2, b2, Wg, bg):
    from concourse.bass_utils import run_bass_kernel_spmd

    Bc = B_FULL // NCORES
    nc = _get_nc()
    in_maps = pack_inputs(
        np.asarray(x),
        np.asarray(W1),
        np.asarray(b1),
        np.asarray(W2),
        np.asarray(b2),
        np.asarray(Wg),
        np.asarray(bg),
        Bc,
        NCORES,
    )
    res = run_bass_kernel_spmd(nc, in_maps, core_ids=list(range(NCORES)))
    return np.concatenate([res.results[c]["out"] for c in range(NCORES)], axis=0)


# revision 29
# speedup vs baseline: 3.8060x; 1.0175x over previous
"""MoE (8 experts, top-5 Boltzmann gate) Trainium2 kernel.

Strategy: data-parallel over tokens. Each of the 8 NeuronCores processes
B/8 = 512 tokens and runs all 8 experts fused (hT stays in SBUF between
the two matmuls); gate weights are applied per-partition at PSUM evict.
No collectives — the host slices tokens and concatenates the outputs.

Host-side prep (sharding/layout): weights are pre-transposed and tiled so
every DMA is contiguous per partition.
"""

import numpy as np

# Problem dims (hardcoded per contract)
D_FULL, H_FULL, O_FULL, NEXP = 1024, 4096, 1024, 8
B_FULL = 4096
NCORES = 8
TEMP = float(np.e)
BIG = 1.0e30
KH_CHUNK = 32  # mm2 contraction tiles per PSUM accumulation group
N_WARMUP_MM = 16  # dependency-free matmuls to bridge input-DMA latency + HAM warm


def build_moe_bass(Bc, D, H, O, N, temp, num_devices=NCORES):
    """Build the per-core Bass/Tile program. Bc = tokens per core (<=512)."""
    from contextlib import ExitStack

    import concourse.bass as bass
    import concourse.tile as tile
    from concourse import bacc, mybir

    f32 = mybir.dt.float32
    # fp16 operands for the heavy matmuls: full-rate PE streaming, FWL weight
    # loads, and half the HBM traffic. PSUM accumulation stays fp32.
    f16 = mybir.dt.float16
    P = 128
    assert Bc % P == 0 and Bc <= 512
    assert D % P == 0 and H % (2 * P) == 0 and O % 512 == 0
    KD, KH, MB, NO = D // P, H // P, Bc // P, O // 512
    MH = H // P
    KH2 = KH // 2  # kh pairs (w2 slab granularity)

    nc = bacc.Bacc(
        "TRN2", target_bir_lowering=False, debug=False, num_devices=num_devices
    )

    # DRAM I/O (host-packed layouts; all per-partition contiguous)
    xt_d = nc.dram_tensor("xt", [P, KD, Bc], f16, kind="ExternalInput").ap()
    xg_d = nc.dram_tensor("xtg", [P, KD, Bc], f32, kind="ExternalInput").ap()
    w1_d = nc.dram_tensor("w1t", [N, MH, P, KD, P], f16, kind="ExternalInput").ap()
    w2_d = nc.dram_tensor("w2t", [N, KH2, P, 2, O], f16, kind="ExternalInput").ap()
    b1_d = nc.dram_tensor("b1p", [P, N, MH], f32, kind="ExternalInput").ap()
    b2_d = nc.dram_tensor("b2s", [N, O], f32, kind="ExternalInput").ap()
    wg_d = nc.dram_tensor("wgt", [P, KD, N], f32, kind="ExternalInput").ap()
    bg_d = nc.dram_tensor("bgr", [P, N], f32, kind="ExternalInput").ap()
    out_d = nc.dram_tensor("out", [Bc, O], f32, kind="ExternalOutput").ap()

    Exp = mybir.ActivationFunctionType.Exp
    Relu = mybir.ActivationFunctionType.Relu
    Alu = mybir.AluOpType

    with tile.TileContext(nc) as tc, ExitStack() as ctx:
        const = ctx.enter_context(tc.tile_pool(name="const", bufs=1))
        gatep = ctx.enter_context(tc.tile_pool(name="gate", bufs=2))
        xtp = ctx.enter_context(tc.tile_pool(name="xt", bufs=1))
        w1p = ctx.enter_context(tc.tile_pool(name="w1", bufs=6))
        w2p = ctx.enter_context(tc.tile_pool(name="w2", bufs=KH // 2 + 2))
        htp = ctx.enter_context(tc.tile_pool(name="ht", bufs=MH + 1))
        accp = ctx.enter_context(tc.tile_pool(name="acc", bufs=MB))
        ps_s = ctx.enter_context(tc.tile_pool(name="ps_s", bufs=2, space="PSUM"))
        ps_1 = ctx.enter_context(tc.tile_pool(name="ps_1", bufs=3, space="PSUM"))
        ps_2 = ctx.enter_context(tc.tile_pool(name="ps_2", bufs=3, space="PSUM"))

        # ---- PE warmup: dependency-free matmuls bridge the input-DMA latency
        # and lift the HAM clock gate before real work arrives.
        wu = const.tile([P, 512], f16, tag="warmup")
        nc.vector.memset(wu[:], 0.0)
        for i in range(N_WARMUP_MM):
            pw = ps_s.tile([P, 512], f32, tag="ps_small", name=f"ps_wu{i}")
            nc.tensor.matmul(pw[:], wu[:, 0:P], wu[:], start=True, stop=True)

        # ---- constant / input loads ----
        # Two HWDGE rings (SP='sync', ACT='scalar'), each FIFO: keep xt + w1
        # on the sync ring so mm1 starts ASAP; consts/xtg/w2 go via scalar.
        xt = xtp.tile([P, KD, Bc], f16)
        nc.sync.dma_start(xt[:], xt_d[:])
        xtg = xtp.tile([P, KD, Bc], f32, tag="xtg")
        nc.scalar.dma_start(xtg[:], xg_d[:])
        wg_sb = const.tile([P, KD, N], f32)
        nc.scalar.dma_start(wg_sb[:], wg_d[:])
        bg_sb = const.tile([P, N], f32)
        nc.scalar.dma_start(bg_sb[:], bg_d[:])
        b1_sb = const.tile([P, N, MH], f32)
        nc.scalar.dma_start(b1_sb[:], b1_d[:])
        b2_sb = const.tile([N, O], f32)
        nc.scalar.dma_start(b2_sb[:], b2_d[:])

        w_sb = const.tile([P, MB, N], f32)  # gate weights, [token_p, mtile, expert]
        wt_sb = const.tile([32, Bc], f32)  # transposed gate weights (rows 0..N-1)

        def emit_mm1(n):
            """hT[m] = relu(W1T_n-tiles.T @ xt + b1) for all H tiles; fp16 out."""
            ht = []
            for m in range(MH):
                w1m = w1p.tile([P, KD, P], f16, tag="w1", name=f"w1m_{n}_{m}")
                nc.sync.dma_start(w1m[:], w1_d[n, m])
                ps1 = ps_1.tile([P, Bc], f32, tag="ps1", name=f"ps1_{n}_{m}")
                for k in range(KD):
                    nc.tensor.matmul(
                        ps1[:],
                        w1m[:, k, :],
                        xt[:, k, :],
                        start=(k == 0),
                        stop=(k == KD - 1),
                    )
                h = htp.tile([P, Bc], f16, tag="ht", name=f"ht_{n}_{m}")
                nc.scalar.activation(h[:], ps1[:], Relu, bias=b1_sb[:, n, m : m + 1])
                ht.append(h)
            return ht

        # expert 0 mm1 first so the PE starts as soon as xt + first w1 land
        ht0 = emit_mm1(0)

        # ---- gate: logits -> softmax(l/T) -> top-5 mask -> renormalize ----
        for m in range(MB):
            pg = ps_s.tile([P, N], f32, tag="ps_small")
            for k in range(KD):
                nc.tensor.matmul(
                    pg[:],
                    xtg[:, k, m * P : (m + 1) * P],
                    wg_sb[:, k, :],
                    start=(k == 0),
                    stop=(k == KD - 1),
                )
            lg = gatep.tile([P, N], f32, tag="g_l")
            nc.vector.tensor_tensor(lg[:], pg[:], bg_sb[:], Alu.add)
            rmax = gatep.tile([P, 1], f32, tag="g_max")
            nc.vector.reduce_max(rmax[:], lg[:], axis=mybir.AxisListType.X)
            nbias = gatep.tile([P, 1], f32, tag="g_nb")
            nc.scalar.mul(nbias[:], rmax[:], -1.0 / temp)
            e = gatep.tile([P, N], f32, tag="g_e")
            nc.scalar.activation(e[:], lg[:], Exp, bias=nbias[:], scale=1.0 / temp)
            z = gatep.tile([P, 1], f32, tag="g_z")
            nc.vector.reduce_sum(z[:], e[:], axis=mybir.AxisListType.X)
            zi = gatep.tile([P, 1], f32, tag="g_zi")
            nc.vector.reciprocal(zi[:], z[:])
            p = gatep.tile([P, N], f32, tag="g_p")
            nc.vector.tensor_scalar_mul(p[:], e[:], zi[:])
            # 3rd-smallest per row via iterated min-masking (drop bottom N-NA=3)
            cur = p
            mn = None
            for r in range(3):
                mn = gatep.tile([P, 1], f32, tag=f"g_mn{r}")
                nc.vector.tensor_reduce(
                    mn[:], cur[:], axis=mybir.AxisListType.X, op=Alu.min
                )
                if r < 2:
                    msk = gatep.tile([P, N], f32, tag=f"g_msk{r}")
                    nc.vector.tensor_scalar(
                        msk[:], cur[:], mn[:], BIG, op0=Alu.is_equal, op1=Alu.mult
                    )
                    nxt = gatep.tile([P, N], f32, tag=f"g_nxt{r}")
                    nc.vector.tensor_tensor(nxt[:], msk[:], cur[:], Alu.max)
                    cur = nxt
            pm = gatep.tile([P, N], f32, tag="g_pm")
            nc.vector.scalar_tensor_tensor(
                pm[:], p[:], mn[:], p[:], op0=Alu.is_gt, op1=Alu.mult
            )
            s = gatep.tile([P, 1], f32, tag="g_s")
            nc.vector.reduce_sum(s[:], pm[:], axis=mybir.AxisListType.X)
            se = gatep.tile([P, 1], f32, tag="g_se")
            nc.vector.tensor_scalar_add(se[:], s[:], 1.0e-8)
            si = gatep.tile([P, 1], f32, tag="g_si")
            nc.vector.reciprocal(si[:], se[:])
            nc.vector.tensor_scalar_mul(w_sb[:, m, :], pm[:], si[:])

            # transpose this m-tile's gate weights into wt_sb[0:N, m*P:(m+1)*P]
            wpad = gatep.tile([P, 32], f32, tag="g_wpad")
            nc.vector.memset(wpad[:], 0.0)
            nc.vector.tensor_copy(wpad[:, 0:N], w_sb[:, m, :])
            for blk in range(4):
                nc.vector.transpose(
                    wt_sb[0:32, m * P + 32 * blk : m * P + 32 * (blk + 1)],
                    wpad[32 * blk : 32 * (blk + 1), 0:32],
                )

        # ---- out_acc init: b2 contribution = w @ b2_stack (K = N experts) ----
        acc = [
            accp.tile([P, O], f32, name=f"acc{m}", tag="acc") for m in range(MB)
        ]
        for m in range(MB):
            for o2 in range(NO):
                pb = ps_s.tile([P, 512], f32, tag="ps_small")
                nc.tensor.matmul(
                    pb[:],
                    wt_sb[0:N, m * P : (m + 1) * P],
                    b2_sb[0:N, o2 * 512 : (o2 + 1) * 512],
                    start=True,
                    stop=True,
                )
                nc.vector.tensor_copy(acc[m][:, o2 * 512 : (o2 + 1) * 512], pb[:])

        # ---- expert loop ----
        n_chunks = (KH + KH_CHUNK - 1) // KH_CHUNK

        def emit_mm2(n, ht):
            """acc[m][:, o2] += w_n * (hT.T @ W2T_n), chunked over kh."""
            for c in range(n_chunks):
                kh_lo = c * KH_CHUNK
                kh_hi = min(KH, kh_lo + KH_CHUNK)
                slabs = {}
                for kh2 in range(kh_lo // 2, (kh_hi + 1) // 2):
                    sl = w2p.tile([P, 2, O], f16, tag="w2", name=f"w2_{n}_{kh2}")
                    nc.scalar.dma_start(sl[:], w2_d[n, kh2])
                    slabs[kh2] = sl
                for m in range(MB):
                    for o2 in range(NO):
                        ps2 = ps_2.tile(
                            [P, 512], f32, tag="ps2", name=f"ps2_{n}_{c}_{m}_{o2}"
                        )
                        for kh in range(kh_lo, kh_hi):
                            nc.tensor.matmul(
                                ps2[:],
                                ht[kh][:, m * P : (m + 1) * P],
                                slabs[kh // 2][:, kh % 2, o2 * 512 : (o2 + 1) * 512],
                                start=(kh == kh_lo),
                                stop=(kh == kh_hi - 1),
                            )
                        a = acc[m][:, o2 * 512 : (o2 + 1) * 512]
                        nc.vector.scalar_tensor_tensor(
                            a,
                            ps2[:],
                            w_sb[:, m, n : n + 1],
                            a,
                            op0=Alu.mult,
                            op1=Alu.add,
                        )

        emit_mm2(0, ht0)
        for n in range(1, N):
            ht = emit_mm1(n)
            emit_mm2(n, ht)

        # ---- store ----
        for m in range(MB):
            nc.sync.dma_start(out_d[m * P : (m + 1) * P, :], acc[m][:])

    nc.compile()
    return nc


def pack_inputs(x, W1, b1, W2, b2, Wg, bg, Bc, ncores):
    """Host-side shard + relayout. Returns per-core input maps."""
    P = 128
    N, H, D = W1.shape
    O = W2.shape[1]
    KD, MH, KH2 = D // P, H // P, H // P // 2

    x = np.ascontiguousarray(x, np.float32)
    # w1t[n, m, p, k, q] = W1[n, m*P+q, k*P+p]  (p = d partition, q = h free)
    w1t = np.ascontiguousarray(
        W1.reshape(N, MH, P, KD, P).transpose(0, 1, 4, 3, 2), np.float16
    )
    w2t = np.ascontiguousarray(
        W2.transpose(0, 2, 1).reshape(N, KH2, 2, P, O).transpose(0, 1, 3, 2, 4),
        np.float16,
    )  # [n, kh2, p, c, o] with value W2[n, o, (kh2*2+c)*P+p]
    b1p = np.ascontiguousarray(
        b1.reshape(N, MH, P).transpose(2, 0, 1), np.float32
    )  # [p, n, m]
    wgt = np.ascontiguousarray(
        Wg.reshape(N, KD, P).transpose(2, 1, 0), np.float32
    )  # [p, k, n]
    bgr = np.ascontiguousarray(np.tile(bg[None, :], (P, 1)), np.float32)
    b2s = np.ascontiguousarray(b2, np.float32)

    in_maps = []
    for c in range(ncores):
        xs = x[c * Bc : (c + 1) * Bc, :]  # [Bc, D]
        xts = np.ascontiguousarray(
            xs.T.reshape(KD, P, Bc).transpose(1, 0, 2), np.float32
        )  # [p, k, b]
        in_maps.append(
            {
                "xt": xts.astype(np.float16),
                "xtg": xts,
                "w1t": w1t,
                "w2t": w2t,
                "b1p": b1p,
                "b2s": b2s,
                "wgt": wgt,
                "bgr": bgr,
            }
        )
    return in_maps


_NC_CACHE = {}


def _get_nc():
    key = (B_FULL // NCORES, D_FULL, H_FULL, O_FULL)
    if key not in _NC_CACHE:
        _NC_CACHE[key] = build_moe_bass(
            B_FULL // NCORES, D_FULL, H_FULL, O_FULL, NEXP, TEMP
        )
    return _NC_CACHE[key]


def kernel(x, W1, b1, W2, b2, Wg, bg):
    from concourse.bass_utils import run_bass_kernel_spmd

    Bc = B_FULL // NCORES
    nc = _get_nc()
    in_maps = pack_inputs(
        np.asarray(x),
        np.asarray(W1),
        np.asarray(b1),
        np.asarray(W2),
        np.asarray(b2),
        np.asarray(Wg),
        np.asarray(bg),
        Bc,
        NCORES,
    )
    res = run_bass_kernel_spmd(nc, in_maps, core_ids=list(range(NCORES)))
    return np.concatenate([res.results[c]["out"] for c in range(NCORES)], axis=0)
